# revision 6
# baseline (speedup 1.0000x reference)
"""Trainium2 Bass kernel for nn_AttnDecoder: Bahdanau-attention LSTM decoder.

Sharding: pure data-parallel over batch (8 rows/core), zero collectives.
  Phase 1: recurrence (attention + LSTM) with h-state kept transposed
           (h on partitions, in Hbuf) so matmuls use activations-stationary
           orientation; tanh(keys + qW1) via DVE broadcast-add + big ACT
           ops; per-step embedding gate contribution precomputed as one
           GEMM before the loop (teacher forcing) into DRAM scratch.
  Phase 2: all T steps' logits as one GEMM streaming outW.T from HBM once;
           log_softmax fully local (full vocab per core): exp with fused
           free-axis accumulation (sumexp), then log(exp * (1/Z)) with
           per-partition scale fused into the evacuation.

Self-contained: hardcodes shapes from the problem spec.
"""
import os
import sys
from contextlib import ExitStack

sys.path.insert(0, "/opt/trn_rl_repo")

import numpy as np
import ml_dtypes

import concourse.bass as bass
import concourse.tile as tile
from concourse import bacc, mybir
from concourse.bass import ds, ts
from concourse.bass_utils import run_bass_kernel_spmd
from concourse.masks import make_identity

AF = mybir.ActivationFunctionType
ALU = mybir.AluOpType
F32 = mybir.dt.float32
BF16 = mybir.dt.bfloat16

# Problem shapes
B, S, T, H, V = 64, 128, 32, 1024, 32000
G = 4 * H          # 4096 gate width
NCORES = 8
BL = B // NCORES   # 8 local batch rows
P = 128
KT = H // P        # 8 k-tiles
ROWS = BL * T      # 256 (t-major: row r = t*BL + b)
HB_COLS = BL * (T + 1)  # Hbuf: col block m holds h_{m-1}; block 0 = h_init
VCH = 500          # phase-2 vocab chunk (psum bank: 500 f32 = 2000B)
NV = V // VCH      # 64
LCH = 2000         # log pass chunk
NL = V // LCH      # 16

COMPUTE = os.environ.get("BASS_CDT", "bfloat16")
CDT = BF16 if COMPUTE == "bfloat16" else F32
NPCDT = ml_dtypes.bfloat16 if COMPUTE == "bfloat16" else np.float32


def _bc(ap, dims):
    """Raw AP with given free dims [[step, count], ...] on ap's tensor."""
    return bass.AP(tensor=ap.tensor, offset=ap.offset, ap=[ap.ap[0]] + dims)


def build_nc():
    nc = bacc.Bacc("TRN2", target_bir_lowering=False, debug=False,
                   num_devices=NCORES)

    def inp(name, shape, dt=CDT):
        return nc.declare_dram_parameter(name, list(shape), dt, isOutput=False)

    def outp(name, shape, dt=F32):
        return nc.declare_dram_parameter(name, list(shape), dt, isOutput=True)

    enc = inp("enc", (S, KT, BL, P))      # [s, ht, b, h_in] batch shard
    enc0T = inp("enc0T", (KT, P, S))      # enc[0].T tiles
    ehT = inp("ehT", (KT, P, BL))
    ecT = inp("ecT", (KT, P, BL))
    W1T = inp("W1T", (KT, P, H))          # [kt, p, j] = W1[j, kt*128+p]
    W2T = inp("W2T", (KT, P, H))
    WhhT = inp("WhhT", (KT, P, G))
    WihcT = inp("WihcT", (KT, P, G))
    WiheT = inp("WiheT", (KT, P, G))
    br1WT = inp("br1WT", (KT, P, H))
    br2WT = inp("br2WT", (KT, P, H))
    outWT = inp("outWT", (KT, P, V))
    VwT = inp("VwT", (P, KT))
    b12 = inp("b12", (P, KT), F32)        # b1 + b2, [p, jt]
    bihh = inp("bihh", (1, G))            # b_ih + b_hh
    br1b = inp("br1b", (1, H))
    br2b = inp("br2b", (1, H))
    outb = inp("outb", (1, V))
    tok = inp("tok", (P, 2), mybir.dt.int32)   # row r = half*128+p -> emb row
    emb = inp("emb", (V, H), F32)

    out_logp = outp("out_logp", (ROWS, V))
    out_h = outp("out_h", (BL, H))
    out_c = outp("out_c", (BL, H))
    out_attn = outp("out_attn", (T, BL, S))

    with tile.TileContext(nc) as tc:
        with ExitStack() as stack:
            persist = stack.enter_context(tc.tile_pool(name="persist", bufs=1))
            dram = stack.enter_context(tc.tile_pool(name="dram", bufs=1,
                                                    space="DRAM"))

            Hbuf = persist.tile([P, KT, HB_COLS], CDT, tag="hbuf")
            ones1 = persist.tile([1, P], CDT, tag="ones1")
            id128 = persist.tile([P, P], F32, tag="id128")
            id8c = persist.tile([BL, BL], CDT, tag="id8c")
            eproj_dram = dram.tile([ROWS, G], CDT)

            nc.vector.memset(ones1[:], 1.0)
            make_identity(nc, id128[:])
            make_identity(nc, id8c[:])

            with ExitStack() as ph1_stack:
                ph1 = ph1_stack.enter_context(
                    tc.tile_pool(name="ph1", bufs=1))
                keysT = ph1.tile([P, KT, S], CDT, tag="keysT")
                VwT_sb = ph1.tile([P, KT], CDT, tag="vwt")
                b12_sb = ph1.tile([P, KT], F32, tag="b12")
                c_hold = ph1.tile([BL, H], F32, tag="c_hold")
                c_tmp = ph1.tile([BL, H], F32, tag="c_tmp")
                Whh_sb = ph1.tile([P, KT, G], CDT, tag="whh")
                Wihc_sb = ph1.tile([P, KT, G], CDT, tag="wihc")

                nc.sync.dma_start(out=VwT_sb[:], in_=VwT[:])
                nc.sync.dma_start(out=b12_sb[:], in_=b12[:])
                for kt in range(KT):
                    nc.sync.dma_start(out=Whh_sb[:, kt, :], in_=WhhT[kt])
                    nc.sync.dma_start(out=Wihc_sb[:, kt, :], in_=WihcT[kt])

                # ---------------- setup: keys ----------------
                with ExitStack() as sc1:
                    sp = sc1.enter_context(tc.tile_pool(name="skeys", bufs=3))
                    kp = sc1.enter_context(
                        tc.tile_pool(name="skeysp", bufs=1, space="PSUM"))
                    e0_sb = sp.tile([P, KT, S], CDT, tag="e0", bufs=1)
                    for kt in range(KT):
                        nc.sync.dma_start(out=e0_sb[:, kt, :], in_=enc0T[kt])
                    psKs = [kp.tile([P, S], F32, tag=f"psK{jt}",
                                    name=f"psK{jt}") for jt in range(KT)]
                    for kt in range(KT):
                        w2 = sp.tile([P, H], CDT, tag="wload")
                        nc.sync.dma_start(out=w2[:], in_=W2T[kt])
                        for jt in range(KT):
                            nc.tensor.matmul(psKs[jt][:], w2[:, ts(jt, P)],
                                             e0_sb[:, kt, :],
                                             start=(kt == 0),
                                             stop=(kt == KT - 1))
                    for jt in range(KT):
                        nc.vector.tensor_scalar_add(keysT[:, jt, :],
                                                    psKs[jt][:],
                                                    b12_sb[:, jt:jt + 1])

                # ---------------- setup: bridge ----------------
                with ExitStack() as sc2:
                    sp = sc2.enter_context(tc.tile_pool(name="sbr", bufs=3))
                    bp = sc2.enter_context(
                        tc.tile_pool(name="sbrp", bufs=2, space="PSUM"))
                    ehT_sb = sp.tile([P, KT, BL], CDT, tag="ehT", bufs=1)
                    ecT_sb = sp.tile([P, KT, BL], CDT, tag="ecT", bufs=1)
                    b1_sb = sp.tile([1, H], CDT, tag="b1b", bufs=1)
                    b2_sb = sp.tile([1, H], CDT, tag="b2b", bufs=1)
                    h0_sb = sp.tile([BL, H], F32, tag="h0", bufs=1)
                    for kt in range(KT):
                        nc.sync.dma_start(out=ehT_sb[:, kt, :], in_=ehT[kt])
                        nc.sync.dma_start(out=ecT_sb[:, kt, :], in_=ecT[kt])
                    nc.sync.dma_start(out=b1_sb[:], in_=br1b[:])
                    nc.sync.dma_start(out=b2_sb[:], in_=br2b[:])

                    for which in range(2):
                        wT = br1WT if which == 0 else br2WT
                        bb = b1_sb if which == 0 else b2_sb
                        src = ehT_sb if which == 0 else ecT_sb
                        dst = h0_sb if which == 0 else c_hold
                        psBs = [bp.tile([BL, 512], F32, tag=f"psB{n}",
                                        name=f"psB{n}") for n in range(2)]
                        for kt in range(KT):
                            w = sp.tile([P, H], CDT, tag="wload")
                            nc.sync.dma_start(out=w[:], in_=wT[kt])
                            for nch in range(2):
                                nc.tensor.matmul(psBs[nch][:], src[:, kt, :],
                                                 w[:, ds(nch * 512, 512)],
                                                 start=(kt == 0), stop=False)
                        for nch in range(2):
                            nc.tensor.matmul(psBs[nch][:], ones1[:, :BL],
                                             bb[:, ds(nch * 512, 512)],
                                             start=False, stop=True)
                            nc.vector.tensor_copy(dst[:, ds(nch * 512, 512)],
                                                  psBs[nch][:])

                    # h0 -> transposed into Hbuf block 0
                    for ht in range(KT):
                        psT = bp.tile([P, BL], F32, tag="psT")
                        nc.tensor.transpose(psT[:], h0_sb[:, ts(ht, P)],
                                            id128[:BL, :BL])
                        nc.vector.tensor_copy(Hbuf[:, ht, 0:BL], psT[:])

                # ---------------- setup: E_proj ----------------
                with ExitStack() as sc3:
                    sp = sc3.enter_context(tc.tile_pool(name="sep", bufs=3))
                    pp = sc3.enter_context(
                        tc.tile_pool(name="sepp", bufs=2, space="PSUM"))
                    tok_sb = sp.tile([P, 2], mybir.dt.int32, tag="tok",
                                     bufs=1)
                    nc.sync.dma_start(out=tok_sb[:], in_=tok[:])
                    embT_sb = sp.tile([P, 2, KT, P], CDT, tag="embT", bufs=1)
                    for half in range(2):
                        embg = sp.tile([P, H], F32, tag="embg", bufs=2)
                        nc.gpsimd.indirect_dma_start(
                            out=embg[:], out_offset=None, in_=emb[:],
                            in_offset=bass.IndirectOffsetOnAxis(
                                ap=tok_sb[:, half:half + 1], axis=0))
                        for kt in range(KT):
                            psE = pp.tile([P, P], F32, tag="psE")
                            nc.tensor.transpose(psE[:], embg[:, ts(kt, P)],
                                                id128[:])
                            nc.vector.tensor_copy(embT_sb[:, half, kt, :],
                                                  psE[:])
                    for nch in range(KT):
                        gsl = ds(nch * 512, 512)
                        bih = sp.tile([1, 512], CDT, tag="bih", bufs=2)
                        nc.sync.dma_start(out=bih[:], in_=bihh[:, gsl])
                        psEPs = [pp.tile([P, 512], F32, tag=f"psEP{hh}",
                                         name=f"psEP{hh}") for hh in range(2)]
                        for kt in range(KT):
                            ch = sp.tile([P, 512], CDT, tag="wchunk")
                            nc.sync.dma_start(out=ch[:], in_=WiheT[kt, :, gsl])
                            for half in range(2):
                                nc.tensor.matmul(psEPs[half][:],
                                                 embT_sb[:, half, kt, :],
                                                 ch[:], start=(kt == 0),
                                                 stop=False)
                        for half in range(2):
                            nc.tensor.matmul(psEPs[half][:], ones1[:],
                                             bih[:], start=False, stop=True)
                            etmp = sp.tile([P, 512], CDT, tag="etmp")
                            nc.vector.tensor_copy(etmp[:], psEPs[half][:])
                            nc.sync.dma_start(
                                out=eproj_dram[ds(half * P, P), gsl],
                                in_=etmp[:])

                # ---------------- recurrence ----------------
                with ExitStack() as lps:
                    work = lps.enter_context(tc.tile_pool(name="work",
                                                          bufs=2))
                    w1ring = lps.enter_context(tc.tile_pool(name="w1ring",
                                                            bufs=3))
                    encring = lps.enter_context(tc.tile_pool(name="encring",
                                                             bufs=3))
                    ps = lps.enter_context(tc.tile_pool(name="lpsum", bufs=2,
                                                        space="PSUM"))
                    psqp = lps.enter_context(tc.tile_pool(name="lpsumq",
                                                          bufs=1,
                                                          space="PSUM"))
                    psgp = lps.enter_context(tc.tile_pool(name="lpsumg",
                                                          bufs=2,
                                                          space="PSUM"))
                    ps1 = lps.enter_context(tc.tile_pool(name="lpsum1",
                                                         bufs=1,
                                                         space="PSUM"))

                    for t in range(T):
                        hcols = ds(t * BL, BL)  # h_{t-1} block in Hbuf
                        # qW1 natural (8, 1024), then transpose to qT
                        psq = psqp.tile([BL, H], F32, tag="psq")
                        for kt in range(KT):
                            w1 = w1ring.tile([P, H], CDT, tag="w1")
                            nc.sync.dma_start(out=w1[:], in_=W1T[kt])
                            for nh in range(2):
                                nc.tensor.matmul(psq[:, ds(nh * 512, 512)],
                                                 Hbuf[:, kt, hcols],
                                                 w1[:, ds(nh * 512, 512)],
                                                 start=(kt == 0),
                                                 stop=(kt == KT - 1))
                        qnat = work.tile([BL, H], F32, tag="hq32", bufs=2,
                                         name="qnat")
                        nc.vector.tensor_copy(qnat[:], psq[:])
                        qT = work.tile([P, KT, BL], CDT, tag="qT")
                        for jt in range(KT):
                            psT = ps.tile([P, BL], F32, tag="tp8")
                            nc.tensor.transpose(psT[:], qnat[:, ts(jt, P)],
                                                id128[:BL, :BL])
                            nc.vector.tensor_copy(qT[:, jt, :], psT[:])

                        # e = tanh(keysT + qT bcast), in 2 batch-halves
                        # scores (1, b, s) += VwT[jt].T @ e[:, :, jt, :]
                        pss = ps1.tile([1, BL, S], F32, tag="pss")
                        for h2 in range(2):
                            bsl = ds(h2 * 4, 4)
                            e_h = work.tile([P, 4, KT, S], CDT, tag="e_h",
                                            bufs=1, name="e_h")
                            keys_bc = _bc(keysT[:], [[0, 4], [S, KT], [1, S]])
                            q_bc = _bc(qT[:, :, bsl], [[1, 4], [BL, KT],
                                                       [0, S]])
                            nc.vector.tensor_tensor(out=e_h[:], in0=keys_bc,
                                                    in1=q_bc, op=ALU.add)
                            nc.scalar.activation(out=e_h[:], in_=e_h[:],
                                                 func=AF.Tanh)
                            for jt in range(KT):
                                nc.tensor.matmul(
                                    pss[:, bsl, :], VwT_sb[:, jt:jt + 1],
                                    e_h[:, :, jt, :],
                                    start=(jt == 0), stop=(jt == KT - 1))

                        # softmax over s (scores small: skip max-subtract)
                        sc = work.tile([1, BL, S], F32, tag="sc", bufs=1,
                                       name="sc")
                        nc.scalar.activation(out=sc[:], in_=pss[:],
                                             func=AF.Exp)
                        sm = work.tile([1, BL], F32, tag="sm")
                        nc.vector.tensor_reduce(out=sm[:], in_=sc[:],
                                                axis=mybir.AxisListType.X,
                                                op=ALU.add)
                        rs = work.tile([1, BL], F32, tag="rs")
                        nc.vector.reciprocal(rs[:], sm[:])
                        rs_bc = _bc(rs[:], [[1, BL], [0, S]])
                        nc.vector.tensor_tensor(out=sc[:], in0=sc[:],
                                                in1=rs_bc, op=ALU.mult)
                        nc.sync.dma_start(out=out_attn[ds(t, 1), :, :],
                                          in_=sc[:])

                        # alphaT (s-part, b)
                        psa = ps.tile([P, BL], F32, tag="tp8")
                        for b in range(BL):
                            nc.tensor.transpose(psa[:, b:b + 1], sc[:, b, :],
                                                id128[:1, :1])
                        aT = work.tile([P, BL], CDT, tag="aT")
                        nc.vector.tensor_copy(aT[:], psa[:])

                        # contextT[ht] (128, b) = enc-tile.T @ aT col
                        ctxT = work.tile([P, KT, BL], CDT, tag="ctxT")
                        for ht in range(KT):
                            ench = encring.tile([S, BL, P], CDT, tag="ench")
                            nc.sync.dma_start(out=ench[:],
                                              in_=enc[:, ht, :, :])
                            psc = ps.tile([P, BL], F32, tag="tp8")
                            for b in range(BL):
                                nc.tensor.matmul(psc[:, b:b + 1],
                                                 ench[:, b, :],
                                                 aT[:, b:b + 1],
                                                 start=True, stop=True)
                            nc.vector.tensor_copy(ctxT[:, ht, :], psc[:])

                        # gates; E_proj folded via id8 matmul; ACT from psum
                        eproj_t = work.tile([BL, G], CDT, tag="eproj",
                                            bufs=1, name="eproj_t")
                        nc.sync.dma_start(out=eproj_t[:],
                                          in_=eproj_dram[ds(t * BL, BL), :])
                        acts = work.tile([BL, G], CDT, tag="acts", bufs=1,
                                         name="acts")
                        for nch in range(KT):
                            psg = psgp.tile([BL, 512], F32, tag="psg")
                            gsl = ds(nch * 512, 512)
                            for kt in range(KT):
                                nc.tensor.matmul(psg[:], Hbuf[:, kt, hcols],
                                                 Whh_sb[:, kt, gsl],
                                                 start=(kt == 0), stop=False)
                            for kt in range(KT):
                                nc.tensor.matmul(psg[:], ctxT[:, kt, :],
                                                 Wihc_sb[:, kt, gsl],
                                                 start=False, stop=False)
                            nc.tensor.matmul(psg[:], id8c[:],
                                             eproj_t[:, gsl],
                                             start=False, stop=True)
                            func = AF.Tanh if nch in (4, 5) else AF.Sigmoid
                            nc.scalar.activation(out=acts[:, gsl], in_=psg[:],
                                                 func=func)

                        # c = sig_f * c + sig_i * tanh_g ; h = sig_o * tanh(c)
                        nc.vector.tensor_tensor(out=c_tmp[:],
                                                in0=acts[:, H:2 * H],
                                                in1=c_hold[:], op=ALU.mult)
                        nc.vector.tensor_tensor(out=acts[:, 0:H],
                                                in0=acts[:, 0:H],
                                                in1=acts[:, 2 * H:3 * H],
                                                op=ALU.mult)
                        nc.vector.tensor_tensor(out=c_hold[:], in0=c_tmp[:],
                                                in1=acts[:, 0:H], op=ALU.add)
                        nc.scalar.activation(out=acts[:, H:2 * H],
                                             in_=c_hold[:], func=AF.Tanh)
                        h_new = work.tile([BL, H], F32, tag="hq32", bufs=2,
                                          name="h_new")
                        nc.vector.tensor_tensor(out=h_new[:],
                                                in0=acts[:, 3 * H:4 * H],
                                                in1=acts[:, H:2 * H],
                                                op=ALU.mult)

                        # h transposed into Hbuf block t+1
                        for ht in range(KT):
                            psT = ps.tile([P, BL], F32, tag="tp8")
                            nc.tensor.transpose(psT[:], h_new[:, ts(ht, P)],
                                                id128[:BL, :BL])
                            nc.vector.tensor_copy(
                                Hbuf[:, ht, ds((t + 1) * BL, BL)], psT[:])

                        if t == T - 1:
                            nc.sync.dma_start(out=out_h[:], in_=h_new[:])
                            nc.sync.dma_start(out=out_c[:], in_=c_hold[:])

            # ---------------- phase 2: logits + log_softmax ----------------
            with ExitStack() as p2:
                pers2 = p2.enter_context(tc.tile_pool(name="pers2", bufs=1))
                ring = p2.enter_context(tc.tile_pool(name="ring", bufs=3))
                lring = p2.enter_context(tc.tile_pool(name="lring", bufs=2))
                psL = p2.enter_context(tc.tile_pool(name="ps2", bufs=4,
                                                    space="PSUM"))

                exp_all = [pers2.tile([P, V], BF16, tag=f"exp{mt}",
                                      name=f"exp{mt}") for mt in range(2)]
                sums = pers2.tile([P, 2, NV], F32, tag="sums")
                rZ = pers2.tile([P, 2], F32, tag="rZ")

                for vch in range(NV):
                    vsl = ds(vch * VCH, VCH)
                    ow = ring.tile([P, KT, VCH], CDT, tag="ow")
                    for kt in range(KT):
                        nc.sync.dma_start(out=ow[:, kt, :],
                                          in_=outWT[kt, :, vsl])
                    ob = ring.tile([1, VCH], CDT, tag="ob")
                    nc.sync.dma_start(out=ob[:], in_=outb[:, vsl])
                    for mt in range(2):
                        pl = psL.tile([P, VCH], F32, tag="pl")
                        for kt in range(KT):
                            nc.tensor.matmul(pl[:],
                                             Hbuf[:, kt, ds(BL + mt * P, P)],
                                             ow[:, kt, :],
                                             start=(kt == 0), stop=False)
                        nc.tensor.matmul(pl[:], ones1[:], ob[:],
                                         start=False, stop=True)
                        nc.scalar.activation(out=exp_all[mt][:, vsl],
                                             in_=pl[:], func=AF.Exp,
                                             accum_out=sums[:, mt,
                                                            vch:vch + 1])

                for mt in range(2):
                    zt = lring.tile([P, 1], F32, tag="zt")
                    nc.vector.tensor_reduce(out=zt[:], in_=sums[:, mt, :],
                                            axis=mybir.AxisListType.X,
                                            op=ALU.add)
                    nc.vector.reciprocal(rZ[:, mt:mt + 1], zt[:])

                for mt in range(2):
                    for lch in range(NL):
                        lsl = ds(lch * LCH, LCH)
                        lp_sb = lring.tile([P, LCH], F32, tag="lp")
                        nc.scalar.activation(out=lp_sb[:],
                                             in_=exp_all[mt][:, lsl],
                                             func=AF.Ln,
                                             scale=rZ[:, mt:mt + 1])
                        nc.sync.dma_start(out=out_logp[ds(mt * P, P), lsl],
                                          in_=lp_sb[:])

    nc.compile()
    return nc


_NC_CACHE = None


def _get_nc():
    global _NC_CACHE
    if _NC_CACHE is None:
        _NC_CACHE = build_nc()
    return _NC_CACHE


def _prep_host(inputs):
    """Shard + lay out inputs for the 8 cores. Returns list of in_maps."""
    f = lambda a: np.asarray(a, np.float32)
    cast = lambda a: np.ascontiguousarray(a).astype(NPCDT)

    enc_full = f(inputs["encoder_outputs"])        # (B, S, H)
    eh = f(inputs["encoder_h"])[0]                 # (B, H)
    ec = f(inputs["encoder_c"])[0]
    tgt = np.asarray(inputs["target_tensor"]).astype(np.int64)   # (B, T)
    emb = f(inputs["emb"])
    W1, b1 = f(inputs["W1"]), f(inputs["b1"])
    W2, b2 = f(inputs["W2"]), f(inputs["b2"])
    Vw = f(inputs["Vw"])
    W_ih, W_hh = f(inputs["W_ih"]), f(inputs["W_hh"])
    b_ih, b_hh = f(inputs["b_ih"]), f(inputs["b_hh"])
    outW, outb = f(inputs["outW"]), f(inputs["outb"])
    br1W, br1b = f(inputs["br1W"]), f(inputs["br1b"])
    br2W, br2b = f(inputs["br2W"]), f(inputs["br2b"])

    def tilesT(WT):  # WT: (H_in, N) k-major -> (KT, P, N)
        return cast(WT.reshape(KT, P, -1))

    shared = {
        "enc0T": cast(enc_full[0].T.reshape(KT, P, S)),
        "W1T": tilesT(W1.T),
        "W2T": tilesT(W2.T),
        "WhhT": tilesT(W_hh.T),
        "WihcT": tilesT(W_ih[:, H:].T),
        "WiheT": tilesT(W_ih[:, :H].T),
        "br1WT": tilesT(br1W.T),
        "br2WT": tilesT(br2W.T),
        "outWT": tilesT(outW.T),
        "VwT": cast(Vw[0].reshape(KT, P).T),
        "b12": np.ascontiguousarray((b1 + b2).reshape(KT, P).T,
                                    dtype=np.float32),
        "bihh": cast((b_ih + b_hh)[None, :]),
        "br1b": cast(br1b[None, :]),
        "br2b": cast(br2b[None, :]),
        "outb": cast(outb[None, :]),
        "emb": emb,
    }
    in_maps = []
    for c in range(NCORES):
        sl = slice(c * BL, (c + 1) * BL)
        r = np.arange(ROWS)
        toks = tgt[c * BL + (r % BL), r // BL].astype(np.int32)
        m = dict(shared)
        # (BL, S, H) -> (S, KT, BL, P)
        esh = enc_full[sl].transpose(1, 0, 2).reshape(S, BL, KT, P)
        m["enc"] = cast(esh.transpose(0, 2, 1, 3))
        m["ehT"] = cast(eh[sl].T.reshape(KT, P, BL))
        m["ecT"] = cast(ec[sl].T.reshape(KT, P, BL))
        m["tok"] = np.ascontiguousarray(toks.reshape(2, P).T)
        in_maps.append(m)
    return in_maps


def kernel(**inputs):
    nc = _get_nc()
    in_maps = _prep_host(inputs)
    res = run_bass_kernel_spmd(nc, in_maps, core_ids=list(range(NCORES)))
    logp = np.empty((B, T, V), np.float32)
    h = np.empty((1, B, H), np.float32)
    c = np.empty((1, B, H), np.float32)
    attn = np.empty((T, B, S), np.float32)
    for ci in range(NCORES):
        r = res.results[ci]
        sl = slice(ci * BL, (ci + 1) * BL)
        logp[sl] = r["out_logp"].reshape(T, BL, V).transpose(1, 0, 2)
        h[0, sl] = r["out_h"]
        c[0, sl] = r["out_c"]
        attn[:, sl] = r["out_attn"]
    return logp, h, c, attn
